# revision 1
# baseline (speedup 1.0000x reference)
"""Trainium2 Bass kernel for nn_MnistPrllSplineKAN.

Math: the pykan spline layer on a uniform extended grid reduces to, per input
dim d, an 8-function cubic B-spline basis phi_j(u) with u = (x+2.2)/0.4 and
integer knots 0..11.  Each phi_j is expanded in truncated cubes
relu(.-i)^3; to keep the features fp-conditioned the domain is split at
S=5.5 and cubes are evaluated on clamped coordinates:

  R_i = relu(min(u,S) - i)^3          i = 0..5    (values <= 166.4)
  L_k = relu(k - max(u,S))^3          k = 6..11   (values <= 166.4)
  phi_j(u) = sum_m beta_m R_{j+m} + sum_m beta_m L_{j+4-m} - phi_j(S)

(beta = (1,-4,6,-4,1)/6; out-of-range terms vanish; the -phi_j(S) constants
fold into a per-(h,o) bias applied at the tanh).  The whole network is then

  y[b,ho]  = tanh( FEAT[b,:] @ W[:,ho] + Cbias[ho] )     K = 784*13 rows
  h1       = tanh( W1.T @ y + b1 )                        per-head 16->8
  out      = W2.T @ h1 + b2                               per-head 8->1

Precision: feature types with large magnitude (R0,R1,L10,L11) run as fp32
matmuls (PE does bf16x2 internally, ~2^-16 effective); the small-magnitude
types and silu run fp16 at full PE rate.  Feature chains are ACT Square
(fused affine) + DVE fused tensor_scalar relu + tensor_tensor multiply:
relu(v)^3 = relu(v) * v^2.

Sharding: pure data parallel, batch 16384 -> 8 cores x 2048.
"""

import numpy as np

B_TOTAL = 16384
N_CORES = 8
B_CORE = B_TOTAL // N_CORES      # 2048
D = 784
HEADS, OUT_DIM = 10, 16
HO = HEADS * OUT_DIM             # 160
NF = 8                           # spline basis functions per dim
NT = 13                          # feature types per dim: 6 R + 6 L + silu
DC = 112                         # d-chunk size (7 * 112 = 784)
NCHUNK = D // DC                 # 7
NBG = 4                          # batch groups of 512
BG = B_CORE // NBG               # 512
S_SPLIT = 5.5

# feature types (t): 0..5 -> R_t, 6..11 -> L_t, 12 -> silu
F32_TYPES = (0, 1, 2, 3, 8, 9, 10, 11)   # large-magnitude cubes -> fp32 matmul

_cache = {}


def _build_weights(coef, scale_base, scale_sp, mask, w1, b1, w2, b2):
    beta = np.array([1.0, -4.0, 6.0, -4.0, 1.0], dtype=np.float64) / 6.0
    eff = (coef * (scale_sp * mask)[..., None]).astype(np.float64)  # [H,D,O,8]
    CR = np.zeros((HEADS, D, OUT_DIM, 6))
    CL = np.zeros((HEADS, D, OUT_DIM, 6))
    for j in range(NF):
        for m in range(5):
            i = j + m
            if i <= 5:
                CR[..., i] += beta[m] * eff[..., j]
            k = j + 4 - m
            if 6 <= k <= 11:
                CL[..., k - 6] += beta[m] * eff[..., j]
    sb = (scale_base * mask).astype(np.float64)                     # [H,D,O]

    def wblock(c, t):
        dsl = slice(c * DC, (c + 1) * DC)
        if t < 6:
            blk = CR[:, dsl, :, t]
        elif t < 12:
            blk = CL[:, dsl, :, t - 6]
        else:
            blk = sb[:, dsl, :]
        return blk.transpose(1, 0, 2).reshape(DC, HO)

    t32 = [t for t in range(NT) if t in F32_TYPES]
    t16 = [t for t in range(NT) if t not in F32_TYPES]
    W32 = np.stack([wblock(c, t) for c in range(NCHUNK) for t in t32], axis=1)
    W16 = np.stack([wblock(c, t) for c in range(NCHUNK) for t in t16], axis=1)
    # constant bias: -sum_d sum_j phi_j(S) * eff[h,d,o,j]
    phiS = np.array(
        [(beta * np.maximum(S_SPLIT - (j + np.arange(5)), 0.0) ** 3).sum() for j in range(NF)]
    )
    Cbias = -np.einsum("j,hdoj->ho", phiS, eff).reshape(HO)
    W1pack = np.zeros((HO, 80), dtype=np.float32)
    for h in range(HEADS):
        for p in range(8):
            for o in range(OUT_DIM):
                W1pack[h * OUT_DIM + o, h * 8 + p] = w1[h, p, o]
    W2pack = np.zeros((80, 16), dtype=np.float32)                   # pad 10 -> 16 cols
    for h in range(HEADS):
        for p in range(8):
            W2pack[h * 8 + p, h] = w2[h, 0, p]
    b1col = b1.reshape(80, 1).astype(np.float32)
    b2col = np.zeros((16, 1), dtype=np.float32)
    b2col[:10, 0] = b2.reshape(10)
    return (
        np.ascontiguousarray(W32.reshape(DC, -1), dtype=np.float32),
        np.ascontiguousarray(W16.reshape(DC, -1), dtype=np.float16),
        Cbias.astype(np.float32),
        W1pack.astype(np.float16),
        b1col,
        W2pack.astype(np.float16),
        b2col,
    )


def _build_nc():
    import concourse.bass as bass
    import concourse.mybir as mybir
    from concourse.tile import TileContext

    f32 = mybir.dt.float32
    f16 = mybir.dt.float16
    Alu = mybir.AluOpType
    Act = mybir.ActivationFunctionType

    n32 = len(F32_TYPES)
    n16 = NT - n32
    t32 = [t for t in range(NT) if t in F32_TYPES]
    t16 = [t for t in range(NT) if t not in F32_TYPES]
    idx32 = {t: i for i, t in enumerate(t32)}
    idx16 = {t: i for i, t in enumerate(t16)}

    nc = bass.Bass(target_bir_lowering=False, debug=True)
    # register the shift constants used as ACT bias values
    for sig in (0.5, 1.5, 2.5, 3.5, 4.5, 5.5):
        t = nc.alloc_sbuf_tensor(f"const-float32-{sig}", [128, 1], f32)
        nc.gpsimd.memset(t.ap(), float(sig))
        nc.const_aps.aps[(f32, float(sig))] = t.ap()
    nc.all_engine_barrier()

    xt = nc.declare_dram_parameter("xt", [D, B_CORE], f32, isOutput=False)
    wp32 = nc.declare_dram_parameter("wp32", [DC, NCHUNK * n32 * HO], f32, isOutput=False)
    wp16 = nc.declare_dram_parameter("wp16", [DC, NCHUNK * n16 * HO], f16, isOutput=False)
    c0b = nc.declare_dram_parameter("c0b", [128, 1], f32, isOutput=False)
    c1b = nc.declare_dram_parameter("c1b", [32, 1], f32, isOutput=False)
    w1a = nc.declare_dram_parameter("w1a", [128, 80], f16, isOutput=False)
    w1b = nc.declare_dram_parameter("w1b", [32, 80], f16, isOutput=False)
    b1c = nc.declare_dram_parameter("b1c", [80, 1], f32, isOutput=False)
    w2p = nc.declare_dram_parameter("w2p", [80, 16], f16, isOutput=False)
    b2c = nc.declare_dram_parameter("b2c", [16, 1], f32, isOutput=False)
    out = nc.declare_dram_parameter("out", [16, B_CORE], f32, isOutput=True)

    with TileContext(nc) as tc:
        with (
            tc.tile_pool(name="cst", bufs=1) as cst,
            tc.tile_pool(name="xin", bufs=2) as xin,
            tc.tile_pool(name="wts", bufs=4) as wts,
            tc.tile_pool(name="ftp", bufs=4) as ftp,
            tc.tile_pool(name="tmp", bufs=3) as tmp,
            tc.tile_pool(name="res", bufs=1) as res,
        ):
            c0b_s = cst.tile([128, 1], f32, name="c0b_s")
            c1b_s = cst.tile([32, 1], f32, name="c1b_s")
            w1a_s = cst.tile([128, 80], f16, name="w1a_s")
            w1b_s = cst.tile([32, 80], f16, name="w1b_s")
            b1c_s = cst.tile([80, 1], f32, name="b1c_s")
            w2p_s = cst.tile([80, 16], f16, name="w2p_s")
            b2c_s = cst.tile([16, 1], f32, name="b2c_s")
            for dst, src in [
                (c0b_s, c0b), (c1b_s, c1b), (w1a_s, w1a), (w1b_s, w1b),
                (b1c_s, b1c), (w2p_s, w2p), (b2c_s, b2c),
            ]:
                nc.gpsimd.dma_start(out=dst[:], in_=src[:])

            y0 = res.tile([128, B_CORE], f16, name="y0")
            y1 = res.tile([32, B_CORE], f16, name="y1")
            out_sb = res.tile([16, B_CORE], f32, name="osb")

            with tc.tile_pool(name="psA", bufs=1, space="PSUM") as psA:
                ps0 = [psA.tile([128, BG], f32, name=f"ps0_{g}") for g in range(NBG)]
                ps1 = [psA.tile([32, BG], f32, name=f"ps1_{g}") for g in range(NBG)]

                for c in range(NCHUNK):
                    xc = xin.tile([DC, B_CORE], f32, name="xc", tag="xc", bufs=2)
                    nc.sync.dma_start(out=xc[:], in_=xt[c * DC : (c + 1) * DC, :])
                    a0 = tmp.tile([DC, B_CORE], f32, name="a0", tag="a0", bufs=2)
                    a1 = tmp.tile([DC, B_CORE], f32, name="a1", tag="a1", bufs=2)
                    # a0 = 2.5*min(x,0), a1 = -2.5*max(x,0); both in (-inf, 0]
                    nc.vector.tensor_scalar(a0[:], xc[:], 0.0, 2.5, Alu.min, Alu.mult)
                    nc.vector.tensor_scalar(a1[:], xc[:], 0.0, -2.5, Alu.max, Alu.mult)
                    for t in range(NT):
                        is32 = t in F32_TYPES
                        fdt = f32 if is32 else f16
                        feat = ftp.tile(
                            [DC, B_CORE], fdt, name=f"feat{'32' if is32 else '16'}",
                            tag=f"feat{'32' if is32 else '16'}", bufs=3,
                        )
                        if t < 12:
                            sig = (S_SPLIT - t) if t < 6 else (t - S_SPLIT)
                            src32 = a0 if t < 6 else a1
                            q = tmp.tile([DC, B_CORE], fdt, name="q", tag=f"q{fdt}", bufs=2)
                            rho = tmp.tile([DC, B_CORE], fdt, name="rho", tag=f"rho{fdt}", bufs=2)
                            nc.scalar.activation(q[:], src32[:], Act.Square, bias=float(sig))
                            nc.vector.tensor_scalar(
                                rho[:], src32[:], float(sig), 0.0, Alu.add, Alu.max
                            )
                            nc.vector.tensor_tensor(feat[:], rho[:], q[:], Alu.mult)
                        else:
                            nc.scalar.activation(feat[:], xc[:], Act.Silu)
                        wchunk = wts.tile([DC, HO], fdt, name="w", tag=f"w{fdt}", bufs=4)
                        if is32:
                            base = (c * n32 + idx32[t]) * HO
                            nc.sync.dma_start(out=wchunk[:], in_=wp32[:, base : base + HO])
                        else:
                            base = (c * n16 + idx16[t]) * HO
                            nc.sync.dma_start(out=wchunk[:], in_=wp16[:, base : base + HO])
                        first = c == 0 and t == 0
                        last = c == NCHUNK - 1 and t == NT - 1
                        for g in range(NBG):
                            fs = feat[:, g * BG : (g + 1) * BG]
                            nc.tensor.matmul(
                                ps0[g][:], wchunk[:, 0:128], fs, start=first, stop=last
                            )
                            nc.tensor.matmul(
                                ps1[g][:], wchunk[:, 128:HO], fs, start=first, stop=last
                            )
                # evacuate with tanh + constant bias
                for g in range(NBG):
                    gs = slice(g * BG, (g + 1) * BG)
                    nc.scalar.activation(y0[:, gs], ps0[g][:], Act.Tanh, bias=c0b_s[:, 0:1])
                    nc.scalar.activation(y1[:, gs], ps1[g][:], Act.Tanh, bias=c1b_s[:, 0:1])

            with tc.tile_pool(name="psB", bufs=2, space="PSUM") as psB:
                for g in range(NBG):
                    gs = slice(g * BG, (g + 1) * BG)
                    h1p = psB.tile([80, BG], f32, name="h1p", tag="h1p", bufs=2)
                    nc.tensor.matmul(h1p[:], w1a_s[:], y0[:, gs], start=True, stop=False)
                    nc.tensor.matmul(h1p[:], w1b_s[:], y1[:, gs], start=False, stop=True)
                    h1 = tmp.tile([80, BG], f16, name="h1", tag="h1", bufs=2)
                    nc.scalar.activation(h1[:], h1p[:], Act.Tanh, bias=b1c_s[:, 0:1])
                    op = psB.tile([16, BG], f32, name="op", tag="op", bufs=2)
                    nc.tensor.matmul(op[:], w2p_s[:], h1[:], start=True, stop=True)
                    nc.scalar.activation(out_sb[:, gs], op[:], Act.Identity, bias=b2c_s[:, 0:1])
            nc.sync.dma_start(out=out[:], in_=out_sb[:])

    _split_wide_waits(nc)
    return nc


def _split_wide_waits(nc, limit=1):
    """walrus here only accepts one sem-wait per instruction; hoist excess
    waits onto no-op Drain carriers inserted before, on the same engine."""
    import bass_rust
    import concourse.mybir as mybir

    ctr = [0]
    for bb in nc.main_func.blocks:
        il = bb.instructions
        i = 0
        while i < len(il):
            ins = il[i]
            si = ins.sync_info
            if si is not None and si.on_wait and len(si.on_wait) > limit:
                waits = list(si.on_wait)
                keep = waits[-limit:]
                extra = waits[:-limit]
                ins.sync_info = bass_rust.SyncInfo(
                    on_wait=keep, on_update=list(si.on_update or [])
                )
                carriers = []
                for j in range(0, len(extra), limit):
                    ctr[0] += 1
                    carriers.append(
                        mybir.InstDrain(
                            name=f"I-waitsplit-{ctr[0]}",
                            engine=ins.engine,
                            ins=[],
                            outs=[],
                            sync_info=bass_rust.SyncInfo(
                                on_wait=extra[j : j + limit], on_update=[]
                            ),
                        )
                    )
                for k, cr in enumerate(carriers):
                    il.insert(i + k, cr)
                i += len(carriers)
            i += 1


def kernel(**inputs):
    x = np.asarray(inputs["x"], dtype=np.float32)
    if "nc" not in _cache:
        _cache["nc"] = _build_nc()
    nc = _cache["nc"]

    if "params" not in _cache:
        _cache["params"] = _build_weights(
            np.asarray(inputs["coef"], np.float64),
            np.asarray(inputs["scale_base"], np.float64),
            np.asarray(inputs["scale_sp"], np.float64),
            np.asarray(inputs["mask"], np.float64),
            np.asarray(inputs["w1"], np.float64),
            np.asarray(inputs["b1"], np.float64),
            np.asarray(inputs["w2"], np.float64),
            np.asarray(inputs["b2"], np.float64),
        )
    W32, W16, Cbias, W1pack, b1col, W2pack, b2col = _cache["params"]
    c0bias = Cbias[:128].reshape(128, 1)
    c1bias = np.ascontiguousarray(Cbias[128:].reshape(32, 1))

    xT = np.ascontiguousarray(x.reshape(B_TOTAL, D).T)  # [784, 16384]
    in_maps = []
    for s in range(N_CORES):
        in_maps.append(
            {
                "xt": np.ascontiguousarray(xT[:, s * B_CORE : (s + 1) * B_CORE]),
                "wp32": W32,
                "wp16": W16,
                "c0b": c0bias,
                "c1b": c1bias,
                "w1a": W1pack[:128],
                "w1b": np.ascontiguousarray(W1pack[128:]),
                "b1c": b1col,
                "w2p": W2pack,
                "b2c": b2col,
            }
        )

    from concourse.bass_utils import run_bass_kernel_spmd

    res = run_bass_kernel_spmd(
        nc, in_maps, list(range(N_CORES)), trace=bool(globals().get("TRACE"))
    )
    if globals().get("TRACE"):
        globals()["LAST_EXEC_NS"] = res.exec_time_ns
    outs = []
    for s in range(N_CORES):
        o = res.results[s]["out"]          # [16, B_CORE]
        outs.append(o[:10].T)              # [B_CORE, 10]
    return np.ascontiguousarray(np.concatenate(outs, axis=0).astype(np.float32))

